# revision 10
# baseline (speedup 1.0000x reference)
"""Block-diagonal projection kernel for Trainium2 (8 NeuronCores, SPMD).

Math: out[b,s,h,o] = sum_i inputs[b,s,h,i] * W[h,o,i]
Shapes: inputs [8, 2048, 16, 128] f32, W [16, 128, 128] f32.

Sharding: data-parallel over batch — core b handles inputs[b] (no
communication).

Precision: all HBM traffic is bf16 (inputs, W, output); the matmul
accumulates in fp32 PSUM. bf16 rounding contributes ~0.5% relative
error, far under the 2e-2 gate, and halves the HBM traffic that
bounds this kernel (fp32 would be 33 MiB/core; bf16 is 16.5 MiB).

Host-side layout prep puts the contraction dim (i) on SBUF partitions
so the device kernel is pure matmul streaming, and pre-chunks the s
axis so every input DMA reads 8 KB-contiguous per-partition lines:
  x per core: [c, i=128, h=16, sc]  (from inputs[b] [s,h,i], s = c*SC+sc)
  w (shared): [i=128, h=16, o=128]  (W.transpose(2,0,1))
Per 128-row s-tile and head h:
  psum[s128, o] = lhsT.T @ rhs, lhsT = x[c][:, h, s128] (stationary,
  [i,128]), rhs = w[:, h, :] ([i, o=128]).  Output lands in natural
[s, h, o] layout, so stores need no transposition anywhere on device.

Schedule: the kernel is bandwidth-bound (~17 MB at the ~425 GB/s
per-core SDMA/fabric rate), so the goal is one gapless DMA stream of
large, efficient transfers:
  - every DMA has >=4 KB contiguous per-partition lines (no quartered
    first transfers — small-line DMAs drag the stream to ~300 GB/s);
  - the read stream (w + 8 input chunks) is issued unconditionally at
    kernel start on the ACT ring and saturates the fabric while the
    compute pipeline spins up;
  - all output tiles go out on the SP ring as soon as their copies
    complete, soaking whatever bandwidth the reads leave, and the
    write backlog keeps the SDMA engines saturated after the reads
    are exhausted — no tail bubble;
  - ALL buffers are resident (8 input chunks + all 16 output tiles,
    132 KB/partition of SBUF), so nothing is ever recycled: input
    DMAs need no waits, and copies are never gated on output DMAs.

PSUM->SBUF cast copies (fp32 -> bf16) are split between DVE
(head-groups 0,1) and ACT (head-groups 2,3) with separate completion
sems (s_cpv / s_cpa) so per-tile completion can be tested without
relying on cross-engine ordering.

Raw-bass engine programs (not Tile): walrus's PE instruction structs
accept at most one sync-wait per instruction, so all cross-engine sync
is standalone wait_ge instructions + then_inc updates:
  SP   : x chunk 0 DMA, then all output DMAs
  ACT  : w DMA, x chunk 1..7 DMAs, then half the PSUM->SBUF copies
  PE   : 4 matmuls per (s-tile, head-group) into one PSUM bank
  DVE  : the other half of the PSUM->SBUF copies
"""

from contextlib import ExitStack

import ml_dtypes
import numpy as np

import concourse.bass as bass
import concourse.mybir as mybir
from concourse.bass_utils import run_bass_kernel_spmd

F32 = mybir.dt.float32
BF16 = mybir.dt.bfloat16
NP_BF16 = ml_dtypes.bfloat16

B, S, H, NI, NO = 8, 2048, 16, 128, 128
N_CORES = 8
SC = 256  # s rows per input chunk (H*NI*SC*2 = 1 MiB per chunk DMA)
CH = S // SC  # 8 chunks
XBUFS = CH  # all input chunks resident in SBUF (8 x 8 KB/partition)
NBANKS = 8  # PSUM banks used (one head-group of 4 matmuls per bank)


def build_nc(s=S, h=H, ni=NI, no=NO, sc=SC):
    assert s % sc == 0 and sc % 128 == 0 and h % 4 == 0
    nt = s // 128  # 128-row s-tiles
    gpt = h // 4  # head-groups per s-tile
    ng = nt * gpt  # total matmul groups
    gpc = (sc // 128) * gpt  # groups per chunk
    ch = s // sc  # chunks
    tpc = sc // 128  # tiles per chunk
    obufs = nt  # all out tiles resident — no recycling anywhere

    # Trim the Bass-constructor preamble: the trailing all-engine barrier
    # and the const-AP memsets cost ~0.5-0.7 us before the first DMA can
    # issue, and nothing in this kernel uses const APs (activation Copy
    # keeps bias as an immediate) or depends on cross-engine ordering at
    # block entry (all cross-engine sync is via semaphores that NEFF
    # init zeroes). Patches are restored before the Block is built, so
    # the block-exit barrier is emitted normally.
    _orig_barrier = bass.Bass.all_engine_barrier
    _orig_memset = bass.BassGpSimd.memset
    bass.Bass.all_engine_barrier = lambda self, *, sem_only=False: None
    bass.BassGpSimd.memset = lambda self, ap, constant: None
    try:
        nc = bass.Bass()
    finally:
        bass.Bass.all_engine_barrier = _orig_barrier
        bass.BassGpSimd.memset = _orig_memset
    x = nc.dram_tensor("x", [ch, ni, h, sc], BF16, kind="ExternalInput")
    w = nc.dram_tensor("w", [ni, h, no], BF16, kind="ExternalInput")
    y = nc.dram_tensor("y", [s, h, no], BF16, kind="ExternalOutput")

    ctx = ExitStack()
    with ctx:
        xts = [ctx.enter_context(nc.sbuf_tensor(f"xt{i}", [ni, h, sc], BF16)) for i in range(XBUFS)]
        ots = [ctx.enter_context(nc.sbuf_tensor(f"ot{i}", [128, h, no], BF16)) for i in range(obufs)]
        wt = ctx.enter_context(nc.sbuf_tensor("wt", [ni, h, no], BF16))
        pss = [ctx.enter_context(nc.psum_tensor(f"ps{i}", [128, 4, no], F32)) for i in range(NBANKS)]
        # Per-chunk DMA-completion sems: two in-flight DMAs incrementing
        # one sem can interleave their 16 per-engine increments, so a
        # shared counter would not say WHICH transfer finished.  Output
        # DMAs share one sem: it is only ever tested for the grand total.
        s_x = [ctx.enter_context(nc.semaphore(f"s_x{i}")) for i in range(ch)]
        s_yd = ctx.enter_context(nc.semaphore("s_yd"))
        s_w = ctx.enter_context(nc.semaphore("s_w"))
        s_pe = ctx.enter_context(nc.semaphore("s_pe"))
        s_cpv = ctx.enter_context(nc.semaphore("s_cpv"))  # DVE copies (gg 0,1)
        s_cpa = ctx.enter_context(nc.semaphore("s_cpa"))  # ACT copies (gg 2,3)
        block = ctx.enter_context(nc.Block())

        @block.sync
        def _(sp):
            sp.dma_start(xts[0][:], x[0]).then_inc(s_x[0], 16)
            for t in range(nt):
                sp.wait_ge(s_cpv, 2 * (t + 1))
                sp.wait_ge(s_cpa, 2 * (t + 1))
                sp.dma_start(y[t * 128 : (t + 1) * 128, :, :], ots[t][:]).then_inc(
                    s_yd, 16
                )
            # data-landed wait for every output DMA
            sp.wait_ge(s_yd, 16 * nt)

        @block.tensor
        def _(pe):
            for g in range(ng):
                t = g // gpt  # s-tile index
                c = t // tpc  # chunk index
                # Waits are consolidated per TILE: every standalone wait_ge
                # drains the PE pipeline, so one pair of copy-done waits
                # covers all 4 banks of the tile (tile t reuses tile t-2's
                # banks).
                if g == 0:
                    pe.wait_ge(s_w, 16)
                    pe.wait_ge(s_x[0], 16)
                elif g % gpt == 0:
                    if g % gpc == 0:
                        pe.wait_ge(s_x[c], 16)
                    if t >= 2:
                        pe.wait_ge(s_cpv, 2 * (t - 1))
                        pe.wait_ge(s_cpa, 2 * (t - 1))
                xt = xts[c]
                t_in_c = t - c * tpc
                ps = pss[g % NBANKS]
                for j in range(4):
                    hh = (g % gpt) * 4 + j
                    mm = pe.matmul(
                        ps[:, j, :],
                        xt[:, hh, t_in_c * 128 : (t_in_c + 1) * 128],
                        wt[:, hh, :],
                        start=(j == 0),
                        stop=(j == 3),
                    )
                mm.then_inc(s_pe, 1)

        @block.vector
        def _(dve):
            for t in range(nt):
                dve.wait_ge(s_pe, gpt * t + 2)
                for gg in (0, 1):
                    dve.tensor_copy(
                        ots[t][:, gg * 4 : (gg + 1) * 4, :],
                        pss[(gpt * t + gg) % NBANKS][:],
                    ).then_inc(s_cpv, 1)

        @block.scalar
        def _(act):
            act.dma_start(wt[:], w[:]).then_inc(s_w, 16)
            for c in range(1, ch):
                act.dma_start(xts[c][:], x[c]).then_inc(s_x[c], 16)
            for t in range(nt):
                act.wait_ge(s_pe, gpt * t + 4)
                for gg in (2, 3):
                    act.copy(
                        ots[t][:, gg * 4 : (gg + 1) * 4, :],
                        pss[(gpt * t + gg) % NBANKS][:],
                    ).then_inc(s_cpa, 1)

    return nc


_NC_CACHE = {}


def _get_nc():
    if "nc" not in _NC_CACHE:
        _NC_CACHE["nc"] = build_nc()
    return _NC_CACHE["nc"]


def run(inputs, W, trace=False):
    """Returns (out [B,S,H,NO] f32, BassKernelResults)."""
    import os

    if trace:
        os.environ.pop("BASS_NEVER_TRACE", None)
    else:
        # The axon NTFF profiling hook module isn't present in this image;
        # make sure a stray BASS_TRACE can't route us onto that path.
        os.environ.setdefault("BASS_NEVER_TRACE", "1")
    inputs = np.asarray(inputs, dtype=np.float32)
    W = np.asarray(W, dtype=np.float32)
    assert inputs.shape == (B, S, H, NI) and W.shape == (H, NO, NI)
    # [b, s, h, i] -> [b, c, sc, h, i] -> [b, c, i, h, sc], cast to bf16
    xh = np.ascontiguousarray(
        inputs.reshape(B, CH, SC, H, NI).transpose(0, 1, 4, 3, 2)
    ).astype(NP_BF16)
    wh = np.ascontiguousarray(W.transpose(2, 0, 1)).astype(NP_BF16)  # [i, h, o]
    in_maps = [{"x": xh[b], "w": wh} for b in range(N_CORES)]
    br = run_bass_kernel_spmd(_get_nc(), in_maps, list(range(N_CORES)), trace=trace)
    out = np.stack([r["y"] for r in br.results]).astype(np.float32)  # [b, s, h, o]
    return out, br


def kernel(inputs, W):
    out, _ = run(inputs, W)
    return out
